# revision 2
# baseline (speedup 1.0000x reference)
"""DiceLoss partial-sum kernel for Trainium2 (8 NeuronCores, data-parallel).

Computes, for input/target of shape (32, 1, 1024, 1024) fp32:
    bin   = (input > 0.5) ? 1.0 : 0.0
    loss1 = 2 * sum(bin * target)
    loss2 = sum(bin) + sum(target)
and returns (loss1, loss2) as fp32 scalars (same structure as the reference).

Sharding: batch dim N=32 is split 4-per-core across 8 cores. Each core
streams its 16 MiB input + 16 MiB target shard through SBUF in [128, 2048]
fp32 tiles and produces per-partition partial sums:
  - DVE scalar_tensor_tensor: out = (in > 0.5) * target, accum -> intersection
  - DVE tensor_scalar:        out = (in > 0.5),          accum -> bin count
  - ACT activation(Copy):     out = target,              accum -> target sum
The three [128, NT] partial-sum blocks are DMA'd out per core and the final
(tiny) reduction over cores/partitions/tiles happens on the host in float64.
"""

import numpy as np

try:
    import concourse.bass  # noqa: F401
except ImportError:  # pragma: no cover - path fallback for bare containers
    import sys

    for _p in ("/opt/trn_rl_repo", "/root/.axon_site/_ro/trn_rl_repo"):
        if _p not in sys.path:
            sys.path.insert(0, _p)

import concourse.bacc as bacc
import concourse.mybir as mybir
import concourse.tile as tile
from concourse.bass_utils import run_bass_kernel_spmd

N_CORES = 8
FULL_SHAPE = (32, 1, 1024, 1024)
FULL_ELEMS = 32 * 1024 * 1024
PER_CORE = FULL_ELEMS // N_CORES  # 4_194_304
P = 128
TILE_F = 2048
NT = PER_CORE // (P * TILE_F)  # 16
THRESH = 0.5

_CACHE: dict = {}


def _build(nt: int, tile_f: int, n_cores: int):
    """Trace + compile the per-core Bass program for a shard of
    nt * 128 * tile_f elements per tensor."""
    f32 = mybir.dt.float32
    nc = bacc.Bacc(
        "TRN2", target_bir_lowering=False, debug=False, num_devices=n_cores
    )
    inp = nc.dram_tensor("input", [nt, P, tile_f], f32, kind="ExternalInput").ap()
    tgt = nc.dram_tensor("target", [nt, P, tile_f], f32, kind="ExternalInput").ap()
    stats = nc.dram_tensor("stats", [P, 3 * nt], f32, kind="ExternalOutput").ap()

    with tile.TileContext(nc) as tc:
        with (
            tc.tile_pool(name="io", bufs=4) as io_pool,
            tc.tile_pool(name="scr", bufs=2) as scr_pool,
            tc.tile_pool(name="st", bufs=1) as st_pool,
        ):
            st = st_pool.tile([P, 3 * nt], f32)
            for t in range(nt):
                ti = io_pool.tile([P, tile_f], f32, tag="ti")
                tt = io_pool.tile([P, tile_f], f32, tag="tt")
                nc.sync.dma_start(out=ti[:], in_=inp[t])
                nc.sync.dma_start(out=tt[:], in_=tgt[t])

                sd = scr_pool.tile([P, tile_f], f32, tag="sd")
                sa = scr_pool.tile([P, tile_f], f32, tag="sa")
                # intersection: (input > 0.5) * target, accumulated per row
                nc.vector.scalar_tensor_tensor(
                    out=sd[:],
                    in0=ti[:],
                    scalar=THRESH,
                    in1=tt[:],
                    op0=mybir.AluOpType.is_gt,
                    op1=mybir.AluOpType.mult,
                    accum_out=st[:, t : t + 1],
                )
                # bin count: (input > 0.5), accumulated per row
                nc.vector.tensor_scalar(
                    out=sd[:],
                    in0=ti[:],
                    scalar1=THRESH,
                    scalar2=None,
                    op0=mybir.AluOpType.is_gt,
                    op1=mybir.AluOpType.add,
                    accum_out=st[:, nt + t : nt + t + 1],
                )
                # target sum on the scalar engine (runs parallel to DVE)
                nc.scalar.activation(
                    out=sa[:],
                    in_=tt[:],
                    func=mybir.ActivationFunctionType.Copy,
                    accum_out=st[:, 2 * nt + t : 2 * nt + t + 1],
                )
            nc.sync.dma_start(out=stats[:], in_=st[:])
    nc.compile()
    return nc


def _get_nc():
    key = (NT, TILE_F, N_CORES)
    if key not in _CACHE:
        _CACHE[key] = _build(*key)
    return _CACHE[key]


def kernel(input: np.ndarray, target: np.ndarray, **run_kwargs):
    inp = np.asarray(input, dtype=np.float32).reshape(N_CORES, NT, P, TILE_F)
    tgt = np.asarray(target, dtype=np.float32).reshape(N_CORES, NT, P, TILE_F)

    nc = _get_nc()
    in_maps = [
        {"input": np.ascontiguousarray(inp[c]), "target": np.ascontiguousarray(tgt[c])}
        for c in range(N_CORES)
    ]
    res = run_bass_kernel_spmd(nc, in_maps, core_ids=list(range(N_CORES)), **run_kwargs)

    inter = 0.0
    binc = 0.0
    tsum = 0.0
    for c in range(N_CORES):
        stats = res.results[c]["stats"].astype(np.float64)
        inter += stats[:, :NT].sum()
        binc += stats[:, NT : 2 * NT].sum()
        tsum += stats[:, 2 * NT :].sum()

    loss1 = np.float32(2.0 * inter)
    loss2 = np.float32(binc + tsum)
    out = (loss1, loss2)
    if run_kwargs.get("trace"):
        return out, res
    return out
